# revision 16
# baseline (speedup 1.0000x reference)
"""v7: raw-Bass (no TileContext) clamp kernel with whole-shard SBUF residency.

Same host-side tiled-planar layout as v6, but the device program is four
hand-sequenced engine streams with 4 semaphores and no tile-pool recycling:
  sync:   y segment loads (2 per tile, split at the column-triple boundary)
  scalar: c segment loads (1 per tile)
  vector: 12 per-column clamp ops per tile (dense step-1 bf16, 2x mode)
  gpsimd: per-column-triple SWDGE stores
The whole shard (78 KB/partition) stays resident, so there are no buffer
reuse waits, no Tile barriers, and a minimal semaphore footprint -- the
Tile version spends ~8.6us before the first DMA packet and ~3.4us after
the last; this program exists to shrink exactly those two windows.
"""

import sys

for _p in ("/opt/trn_rl_repo", "/root/.axon_site/_ro/trn_rl_repo"):
    if _p not in sys.path:
        sys.path.append(_p)

import numpy as np
import ml_dtypes

_P = 128
_TPP = 3908          # padded +1 row so every tile is even-sized: all DVE
_S = _P * _TPP       # column slices stay 4-byte aligned (packed-mode safe)
_NCORES = 8
_T_LIST = [1024, 1024, 1024, 836]

_PROG_CACHE = {}


def _build_program(t_list):
    from concourse import bacc, mybir

    tpp = sum(t_list)
    bf16 = mybir.dt.bfloat16

    nc = bacc.Bacc("TRN2", target_bir_lowering=False, debug=False,
                   num_devices=_NCORES)
    y_d = nc.dram_tensor("y", (_P, 6 * tpp), bf16, kind="ExternalInput").ap()
    c_d = nc.dram_tensor("c", (_P, 4 * tpp), bf16, kind="ExternalInput").ap()
    o_d = nc.dram_tensor("o", (_P, 6 * tpp), bf16, kind="ExternalOutput").ap()

    y_s = nc.alloc_sbuf_tensor("ybuf", (_P, 6 * tpp), bf16).ap()
    c_s = nc.alloc_sbuf_tensor("cbuf", (_P, 4 * tpp), bf16).ap()

    # One semaphore per load DMA: per-engine completion increments from
    # consecutive DMAs on one queue interleave, so intermediate thresholds
    # on a shared sem would not mean "DMA k done" (CoreSim flags this).
    sem_y = [nc.alloc_semaphore(f"sem_y{i}") for i in range(2 * len(t_list))]
    sem_c = [nc.alloc_semaphore(f"sem_c{i}") for i in range(len(t_list))]
    sem_v = nc.alloc_semaphore("sem_v")
    sem_o = nc.alloc_semaphore("sem_o")

    # Load streams: no waits at all -- buffers are written exactly once.
    # (Measured: y on sync / c on scalar with 4 tiles runs the DMA window
    # gapless at ~401 GB/s; shifting the last tile's y to the scalar ring
    # with a 5-tile list regressed 49.7us -> 60us, so keep this shape.)
    r0 = 0
    for k, t in enumerate(t_list):
        y0, c0 = 6 * r0, 4 * r0
        nc.sync.dma_start(y_s[:, y0:y0 + 3 * t],
                          y_d[:, y0:y0 + 3 * t]).then_inc(sem_y[2 * k], 16)
        nc.sync.dma_start(y_s[:, y0 + 3 * t:y0 + 6 * t],
                          y_d[:, y0 + 3 * t:y0 + 6 * t]).then_inc(
                              sem_y[2 * k + 1], 16)
        nc.scalar.dma_start(c_s[:, c0:c0 + 4 * t],
                            c_d[:, c0:c0 + 4 * t]).then_inc(sem_c[k], 16)
        r0 += t

    # Compute stream. The DVE's SBUF writes retire with pipelined latency,
    # so both the in-place min->max chain and the max->store handoff are
    # fenced with engine DRAINs (wait until every outstanding write has
    # landed) -- a hardware guarantee, unlike per-op @complete sem chains
    # which left a ~1-element intermittent race on HW.
    r0 = 0
    for k, t in enumerate(t_list):
        y3 = y_s[:, 6 * r0:6 * (r0 + t)].rearrange("p (d q) -> p d q", d=6)
        c3 = c_s[:, 4 * r0:4 * (r0 + t)].rearrange("p (d q) -> p d q", d=4)
        nc.vector.wait_ge(sem_c[k], 16)
        for half, (d0, lo_p) in enumerate(((0, 0), (3, 2))):
            nc.vector.wait_ge(sem_y[2 * k + half], 16)
            for d in range(d0, d0 + 3):
                col = y3[:, d, :]
                nc.vector.tensor_tensor(
                    col, col, c3[:, lo_p + 1, :], mybir.AluOpType.min)
            nc.vector.drain()
            for d in range(d0, d0 + 3):
                col = y3[:, d, :]
                nc.vector.tensor_tensor(
                    col, col, c3[:, lo_p, :], mybir.AluOpType.max)
            nc.vector.drain().then_inc(sem_v, 1)
        r0 += t

    # Store stream: one SWDGE store per column-triple, gated on the drained
    # (fully retired) max results of that half.
    r0 = 0
    n_stores = 0
    for k, t in enumerate(t_list):
        y0 = 6 * r0
        for half, d0 in enumerate((0, 3)):
            nc.gpsimd.wait_ge(sem_v, 2 * k + half + 1)
            nc.gpsimd.dma_start(
                o_d[:, y0 + d0 * t:y0 + (d0 + 3) * t],
                y_s[:, y0 + d0 * t:y0 + (d0 + 3) * t]).then_inc(sem_o, 16)
            n_stores += 1
        r0 += t

    nc.gpsimd.wait_ge(sem_o, 16 * n_stores)

    nc.compile()
    return nc


def _get_program():
    key = ("raw", tuple(_T_LIST))
    if key not in _PROG_CACHE:
        _PROG_CACHE[key] = _build_program(_T_LIST)
    return _PROG_CACHE[key]


def _tile_pack(shard2, t_list, width):
    tpp = sum(t_list)
    a = shard2.reshape(_P, tpp, width)
    blocks = []
    r0 = 0
    for t in t_list:
        blocks.append(np.ascontiguousarray(
            a[:, r0:r0 + t, :].transpose(0, 2, 1)).reshape(_P, width * t))
        r0 += t
    return np.concatenate(blocks, axis=1)


def _tile_unpack_f32(dev, t_list, width):
    tpp = sum(t_list)
    out = np.empty((_P, tpp, width), dtype=np.float32)
    c0 = 0
    r0 = 0
    for t in t_list:
        blk = np.asarray(dev[:, c0:c0 + width * t]).astype(np.float32)
        out[:, r0:r0 + t, :] = blk.reshape(_P, width, t).transpose(0, 2, 1)
        c0 += width * t
        r0 += t
    return out.reshape(_P * tpp, width)


def _make_in_maps(y_pred, constr_para):
    y_b = np.ascontiguousarray(y_pred, dtype=np.float32).astype(
        ml_dtypes.bfloat16)
    c_b = np.ascontiguousarray(constr_para, dtype=np.float32).astype(
        ml_dtypes.bfloat16)
    batch = y_pred.shape[0]
    offs = [min(i * _S, batch - _S) for i in range(_NCORES)]
    in_maps = [
        {"y": _tile_pack(y_b[o:o + _S], _T_LIST, 6),
         "c": _tile_pack(c_b[o:o + _S], _T_LIST, 4)} for o in offs
    ]
    return in_maps, offs


def kernel(y_pred: np.ndarray, constr_para: np.ndarray) -> np.ndarray:
    from concourse.bass_utils import run_bass_kernel_spmd

    batch = y_pred.shape[0]
    in_maps, offs = _make_in_maps(y_pred, constr_para)

    nc = _get_program()
    res = run_bass_kernel_spmd(nc, in_maps, core_ids=list(range(_NCORES))).results

    out = np.empty((batch, 6), dtype=np.float32)
    for o, r in zip(offs, res):
        out[o:o + _S] = _tile_unpack_f32(r["o"], _T_LIST, 6)
    return out
